# revision 35
# baseline (speedup 1.0000x reference)
"""Trainium2 Bass kernel for a causal dense-transformer attention layer.

Reference computation (b=4, s=2048, d=1024, 16 heads, dh=64):
  qkv = x0 @ W_in ; causal softmax attention ; out = attn @ W_o
  y = LayerNorm(out + x0)   (no affine, eps=1e-5)

Sharding over 8 cores: core = (batch bi = core//2, head-group tp = core%2).
Each core computes QKV projection + attention for its 8 heads of one batch
(tensor parallel over head groups), then the output projection partial sums
are pair-ReduceScattered so residual + LayerNorm run locally on each core's
1024 output rows.

v2 layout notes:
- scores are computed transposed (keys on partitions, queries free); both
  heads of a pair write adjacent PSUM banks of one [128,1024] tile so a
  single ACT exp instruction covers them (the Scalar engine is the pacing
  resource in the attention phase).
- softmax denominators ride as a ones-column inside V (PSUM row 64); they
  are normalized once per query-chunk: one batched reciprocal on an [8,512]
  gather, broadcast back via DRAM-roundtrip gpsimd DMAs.
- program order starts attention for head-pair 0 right after its Q/K
  projection so exp starts ~25us in; out-projection blocks are issued per
  128-row block as soon as their query-chunk is normalized, leaving only
  two small ReduceScatters exposed at the end.
"""

import os
import sys
from contextlib import ExitStack

import numpy as np

for _p in ("/opt/trn_rl_repo",):
    if os.path.isdir(_p) and _p not in sys.path:
        sys.path.insert(0, _p)

import concourse.bass as bass
import concourse.tile as tile
from concourse import bacc
from concourse import mybir
from concourse.bass_utils import run_bass_kernel_spmd

B, S, D = 4, 2048, 1024
NH, DH = 16, 64
HL = NH // 2          # heads per core
SH = S // 2           # output seq rows per core
NCORES = 8
SCALE = DH ** -0.5    # 0.125
LN_EPS = 1e-5

F16 = mybir.dt.float16
F32 = mybir.dt.float32
Exp = mybir.ActivationFunctionType.Exp
Ln = mybir.ActivationFunctionType.Ln

# out-proj chunk c -> the four 128-row q blocks it carries (2 low, 2 high)
CHUNKS = [[0, 1, 8, 9], [2, 3, 10, 11], [4, 5, 12, 13], [6, 7, 14, 15]]


def build_nc():
    nc = bacc.Bacc("TRN2", target_bir_lowering=False, num_devices=NCORES)
    xT = nc.declare_dram_parameter("xT", [D, S], F16, isOutput=False)
    wqk = nc.declare_dram_parameter("wqk", [D, 2 * HL * DH], F16, isOutput=False)
    wv = nc.declare_dram_parameter("wv", [D, HL * DH], F16, isOutput=False)
    wo = nc.declare_dram_parameter("wo", [HL * DH, D], F16, isOutput=False)
    xres = nc.declare_dram_parameter("xres", [SH, D], F32, isOutput=False)
    # [T | T]: the 128x128 causal triangle (k<=q), duplicated for both heads
    cmsk = nc.declare_dram_parameter("cmask", [128, 256], F16, isOutput=False)
    out = nc.declare_dram_parameter("out", [SH, D], F32, isOutput=True)

    with tile.TileContext(nc, num_cores=NCORES) as tc, ExitStack() as top:
        persist = top.enter_context(tc.tile_pool(name="persist", bufs=1))
        # QT rows 0..511 (tiles 0-3, head pair t on tile t), KT rows 512..1023
        qkt = [persist.tile([128, S], F16, name=f"qkt{m}") for m in range(8)]
        # V in (seq-part, head*dh free) orientation + trailing ones column
        vsb = [persist.tile([128, HL * (DH + 1)], F16, name=f"vsb{m}") for m in range(16)]
        # normalized attn-out^T (head*dh on partitions, seq free)
        aot = [persist.tile([128, S], F16, name=f"aot{t}") for t in range(4)]
        cm = persist.tile([128, 256], F16, name="cm")
        eps_t = persist.tile([128, 1], F32, name="eps_t")
        nc.vector.memset(eps_t, LN_EPS)
        for m in range(16):
            vones = vsb[m].rearrange("p (h c) -> p h c", c=DH + 1)[:, :, DH:DH + 1]
            nc.vector.memset(vones, 1.0)
        nc.sync.dma_start(out=cm, in_=cmsk[:, :])

        # attention pools open first so their PSUM banks never alias the
        # projection PSUM pool (pools are a strict stack; proj closes first)
        attn_ctx = ExitStack()
        adram = attn_ctx.enter_context(tc.tile_pool(name="adram", bufs=2, space="DRAM"))
        asb = attn_ctx.enter_context(tc.tile_pool(name="asb", bufs=5))
        scps = attn_ctx.enter_context(tc.tile_pool(name="scps", bufs=2, space="PSUM"))
        avps = attn_ctx.enter_context(tc.tile_pool(name="avps", bufs=1, space="PSUM"))
        small = attn_ctx.enter_context(tc.tile_pool(name="small", bufs=2))

        proj_ctx = ExitStack()
        proj_in = proj_ctx.enter_context(tc.tile_pool(name="proj_in", bufs=1))
        pjps = proj_ctx.enter_context(tc.tile_pool(name="pjps", bufs=2, space="PSUM"))
        xt = [proj_in.tile([128, S], F16, name=f"xt{k}") for k in range(8)]
        wqs = [proj_in.tile([128, 2 * HL * DH], F16, name=f"wqs{k}") for k in range(8)]
        wvs = [proj_in.tile([128, HL * DH], F16, name=f"wvs{k}") for k in range(8)]
        # interleave so the k-accumulation stream can start on first arrivals
        # spread input loads across three DMA queues so the projection's
        # first k-slices arrive as early as possible
        for k in range(8):
            nc.sync.dma_start(out=xt[k], in_=xT[k * 128:(k + 1) * 128, :])
            nc.gpsimd.dma_start(out=wqs[k], in_=wqk[k * 128:(k + 1) * 128, :])
            nc.scalar.dma_start(out=wvs[k], in_=wv[k * 128:(k + 1) * 128, :])

        # ~3.5us of junk matmuls on the (tiny, early-loaded) mask tile trip
        # the HAM clock gate to full rate before the real lead-in arrives
        wup = pjps.tile([128, 512], F32, tag="pj", name="wup")
        for _ in range(30):
            nc.tensor.matmul(wup[:, 0:256], cm[:, 0:128], cm[:, 0:256],
                             start=True, stop=True)
        wud = small.tile([1, 4], F32, tag="wud", name="wud", bufs=2)
        nc.vector.tensor_copy(wud, wup[0:1, 0:4])

        def proj_v(m):
            ps = pjps.tile([128, 512], F32, tag="pj", name="pjv")
            for k in range(8):
                nc.tensor.matmul(ps, xt[k][:, m * 128:(m + 1) * 128], wvs[k],
                                 start=(k == 0), stop=(k == 7))
            vdst = vsb[m].rearrange("p (h c) -> p h c", c=DH + 1)[:, :, 0:DH]
            nc.vector.tensor_copy(vdst, ps.rearrange("p (h c) -> p h c", c=DH))

        def proj_qk_g(m, q4):  # one 512-column group of Q/K tile m
            ps = pjps.tile([128, 512], F32, tag="pj", name="pjqk")
            for k in range(8):
                nc.tensor.matmul(ps, wqs[k][:, m * 128:(m + 1) * 128],
                                 xt[k][:, q4 * 512:(q4 + 1) * 512],
                                 start=(k == 0), stop=(k == 7))
            nc.vector.tensor_copy(qkt[m][:, q4 * 512:(q4 + 1) * 512], ps)

        def proj_qk(m):
            for q4 in range(4):
                proj_qk_g(m, q4)

        # lead-in: only what attention chunk (t=0, qc=0) needs at normal
        # priority; the rest of the projection is demoted so the scheduler
        # treats it as gap-filler under the ACT-paced attention stream
        # (dependencies still pull each piece in before its consumer)
        proj_qk_g(0, 0); proj_qk_g(4, 0)
        proj_v(0); proj_v(1); proj_v(2); proj_v(3)
        with tc.high_priority(offset=-800):
            for q4 in range(1, 4):
                proj_qk_g(0, q4); proj_qk_g(4, q4)
            proj_qk(1); proj_qk(5)
            proj_v(4); proj_v(5); proj_v(6); proj_v(7)
            proj_qk(2); proj_qk(6)
            proj_v(8); proj_v(9); proj_v(10); proj_v(11)
            proj_qk(3); proj_qk(7)
            proj_v(12); proj_v(13); proj_v(14); proj_v(15)

        w = DH + 1

        cmh = cm.rearrange("p (h c) -> p h c", h=2)

        def attn_chunk(t, qlo, qw, nkb, qdn):
            q_t, k_t = qkt[t], qkt[4 + t]
            dlo = qlo // 128  # first diagonal key block index
            HP = 512          # head pitch: keeps all matmul PSUM dsts bank-aligned
            av = avps.tile([65, 2 * HP], F32, tag="av", name="av")
            for kb in range(nkb):
                ksl = slice(kb * 128, (kb + 1) * 128)
                r = kb - dlo
                v0 = 128 * r if r > 0 else 0  # first causally-valid column
                vw = qw - v0
                sp = scps.tile([128, 2 * HP], F32, tag="sp", name="sp")
                # scores^T = K^T.T @ Q^T; the two heads land in adjacent
                # PSUM banks and use disjoint PE row groups (h0 / h64);
                # diagonal blocks write only the valid columns, compacted to
                # the bank start (matmul PSUM dsts must be bank-aligned)
                nc.tensor.matmul(sp[:, 0:vw], k_t[0:64, ksl],
                                 q_t[0:64, qlo + v0:qlo + qw], start=True, stop=True)
                nc.tensor.matmul(sp[:, HP:HP + vw], k_t[64:128, ksl],
                                 q_t[64:128, qlo + v0:qlo + qw], start=True, stop=True)
                ep = asb.tile([128, 2 * HP], F16, tag="ep", name="ep", bufs=5)
                eph = ep.rearrange("p (h q) -> p h q", h=2)
                sph = sp.rearrange("p (h q) -> p h q", h=2)
                # exp de-compacts: reads [0:vw], writes at [v0:qw]
                nc.scalar.activation(eph[:, :, v0:qw], sph[:, :, 0:vw], Exp, scale=SCALE)
                if r >= 0:  # diagonal 128-col block: apply causal triangle
                    if v0 > 0:
                        nc.vector.memset(eph[:, :, 0:v0], 0.0)
                    nc.vector.tensor_mul(eph[:, :, v0:v0 + 128],
                                         eph[:, :, v0:v0 + 128], cmh)
                st, sp_ = (kb == 0), (kb == nkb - 1)
                # attn-out^T accumulation; V carries a trailing ones column,
                # so the softmax denominator accumulates into PSUM row 64
                nc.tensor.matmul(av[:, 0:qw], vsb[kb][:, (2 * t) * w:(2 * t + 1) * w],
                                 ep[:, 0:qw], start=st, stop=sp_)
                nc.tensor.matmul(av[:, HP:HP + qw],
                                 vsb[kb][:, (2 * t + 1) * w:(2 * t + 2) * w],
                                 ep[:, HP:HP + qw], start=st, stop=sp_)
            # drain PSUM: rows 0..63 data, row 64 denominator
            au = small.tile([128, 512], F32, tag="au", name="au", bufs=10)
            stg = small.tile([65, 512], F32, tag="stg", name="stg", bufs=3)
            nc.vector.tensor_copy(au[0:65, 0:qw], av[:, 0:qw])
            nc.vector.tensor_copy(stg[:, 0:qw], av[:, HP:HP + qw])
            nc.sync.dma_start(out=qdn[2 * t:2 * t + 1, 0:qw], in_=au[64:65, 0:qw])
            nc.sync.dma_start(out=qdn[2 * t + 1:2 * t + 2, 0:qw], in_=stg[64:65, 0:qw])
            nc.sync.dma_start(out=au[64:128, 0:qw], in_=stg[0:64, 0:qw])
            return au

        def norm_qc(qlo, qw, qdn, aus):
            # one reciprocal for all 8 denominator rows of this query chunk,
            # then broadcast across partitions via DRAM-roundtrip DMA
            dn = small.tile([8, 512], F32, tag="dn", name="dn", bufs=2)
            nc.gpsimd.dma_start(out=dn[:, 0:qw], in_=qdn[:, 0:qw])
            dnr = small.tile([8, 512], F32, tag="dnr", name="dnr", bufs=2)
            # ~51-ULP approx reciprocal: ~5x faster than reciprocal(), far
            # more accurate than the softmax denominator needs, and short
            # enough not to head-of-line-block the DVE queue
            nc.vector.reciprocal_approx_fast(out=dnr[:, 0:qw], in_=dn[:, 0:qw])
            rdn = adram.tile([8, 512], F32, tag="rdn", name="rdn", bufs=2)
            nc.sync.dma_start(out=rdn[:, 0:qw], in_=dnr[:, 0:qw])
            for t in range(4):
                rb = small.tile([128, 512], F32, tag="rb", name="rb", bufs=2)
                for j in range(2):
                    srow = rdn[2 * t + j:2 * t + j + 1, 0:qw]
                    bc = bass.AP(tensor=srow.tensor, offset=srow.offset,
                                 ap=[[0, 64], [1, qw]])
                    nc.gpsimd.dma_start(out=rb[j * 64:(j + 1) * 64, 0:qw], in_=bc)
                with tc.high_priority(offset=-400):
                    nc.vector.tensor_mul(aot[t][:, qlo:qlo + qw], aus[t][:, 0:qw],
                                         rb[:, 0:qw])

        def run_qc(qlo, qw):
            nkb = (qlo + qw) // 128
            qdn = adram.tile([8, 512], F32, tag="qdn", name="qdn", bufs=2)
            aus = [attn_chunk(t, qlo, qw, nkb, qdn) for t in range(4)]
            norm_qc(qlo, qw, qdn, aus)

        # qc0 and qc1 interleaved t-major: each head pair's Q/K projection
        # unlock feeds 12 key-blocks of exp work instead of 4, so the ACT
        # stream stays fed while the demoted projection fills PE gaps
        qdn0 = adram.tile([8, 512], F32, tag="qdn", name="qdn", bufs=2)
        qdn1 = adram.tile([8, 512], F32, tag="qdn", name="qdn", bufs=2)
        aus0, aus1 = [], []
        for t in range(4):
            aus0.append(attn_chunk(t, 0, 512, 4, qdn0))
            aus1.append(attn_chunk(t, 512, 512, 8, qdn1))
        norm_qc(0, 512, qdn0, aus0)
        norm_qc(512, 512, qdn1, aus1)
        proj_ctx.close()

        fin = ExitStack()
        dpool = fin.enter_context(tc.tile_pool(name="dram", bufs=1, space="DRAM"))
        fsb = fin.enter_context(tc.tile_pool(name="fsb", bufs=1))
        fps = fin.enter_context(tc.tile_pool(name="fps", bufs=2, space="PSUM"))
        lnp = fin.enter_context(tc.tile_pool(name="lnp", bufs=2))

        wos = [fsb.tile([128, D], F16, name=f"wos{k}") for k in range(4)]
        for k in range(4):
            nc.sync.dma_start(out=wos[k], in_=wo[k * 128:(k + 1) * 128, :])

        rs_in = [dpool.tile([512, D], F16, name=f"rs_in{c}", bufs=4) for c in range(4)]
        rs_out = [dpool.tile([256, D], F16, name=f"rs_out{c}", bufs=4) for c in range(4)]

        def out_j(c, j):
            m = CHUNKS[c][j]
            pstg = lnp.tile([128, D], F16, tag="pstg", name="pstg")
            for n2 in range(2):
                po = fps.tile([128, 512], F32, tag="po", name="po")
                for k in range(4):
                    nc.tensor.matmul(po, aot[k][:, m * 128:(m + 1) * 128],
                                     wos[k][:, n2 * 512:(n2 + 1) * 512],
                                     start=(k == 0), stop=(k == 3))
                nc.vector.tensor_copy(pstg[:, n2 * 512:(n2 + 1) * 512], po)
            nc.sync.dma_start(out=rs_in[c][j * 128:(j + 1) * 128, :], in_=pstg)

        def rs_c(c):
            nc.gpsimd.collective_compute(
                "ReduceScatter", mybir.AluOpType.add,
                replica_groups=[[0, 1], [2, 3], [4, 5], [6, 7]],
                ins=[rs_in[c].opt()], outs=[rs_out[c].opt()])

        def ln_c(c):
            for j in range(2):
                m = 2 * c + j
                y = lnp.tile([128, D], F32, tag="y", name="y")
                yin = lnp.tile([128, D], F16, tag="yin", name="yin")
                xr = lnp.tile([128, D], F32, tag="xr", name="xr")
                nc.sync.dma_start(out=xr, in_=xres[m * 128:(m + 1) * 128, :])
                nc.scalar.dma_start(out=yin, in_=rs_out[c][j * 128:(j + 1) * 128, :])
                nc.vector.tensor_add(y, yin, xr)
                stats = lnp.tile([128, 2, 6], F32, tag="st", name="st")
                mv = lnp.tile([128, 2], F32, tag="mv", name="mv")
                for sg in range(2):
                    nc.vector.bn_stats(out=stats[:, sg, :], in_=y[:, sg * 512:(sg + 1) * 512])
                nc.vector.bn_aggr(out=mv, in_=stats)
                # rstd = rsqrt(var+eps) via DVE-only Newton iteration (the
                # ACT sqrt lives in a different table set than exp, and each
                # switch evicts the attention exp tables for ~2.7us)
                vv = lnp.tile([128, 1], F32, tag="vv", name="vv")
                nc.vector.tensor_scalar(out=vv, in0=mv[:, 1:2], scalar1=LN_EPS,
                                        scalar2=None, op0=mybir.AluOpType.add)
                rstd = lnp.tile([128, 1], F32, tag="rs", name="rs")
                tn = lnp.tile([128, 1], F32, tag="tn", name="tn")
                nc.vector.reciprocal(rstd, vv)  # r = 1/v
                # seed 0.675*r + 0.3 (<=10% err for v in [0.95, 8.3]), then
                # 3x y *= 1.5 - 0.5*v*y^2 -> ~1e-7 rel err
                nc.vector.tensor_scalar(out=rstd, in0=rstd, scalar1=0.675,
                                        scalar2=0.3, op0=mybir.AluOpType.mult,
                                        op1=mybir.AluOpType.add)
                for _ in range(3):
                    nc.vector.tensor_mul(tn, rstd, rstd)
                    nc.vector.tensor_mul(tn, tn, vv)
                    nc.vector.tensor_scalar(out=tn, in0=tn, scalar1=-0.5,
                                            scalar2=1.5, op0=mybir.AluOpType.mult,
                                            op1=mybir.AluOpType.add)
                    nc.vector.tensor_mul(rstd, rstd, tn)
                ot = lnp.tile([128, D], F32, tag="ot", name="ot")
                nc.vector.tensor_scalar(out=ot, in0=y, scalar1=mv[:, 0:1], scalar2=rstd,
                                        op0=mybir.AluOpType.subtract,
                                        op1=mybir.AluOpType.mult)
                nc.sync.dma_start(out=out[m * 128:(m + 1) * 128, :], in_=ot)

        # q blocks 0..7 (chunk lows) can project as soon as qc0/qc1 land
        # q blocks 0..7 (chunk lows) fill PE gaps under qc2's attention
        with tc.high_priority(offset=-400):
            for c in range(4):
                out_j(c, 0)
                out_j(c, 1)
        run_qc(1024, 512)
        out_j(0, 2); out_j(0, 3)
        rs_c(0); ln_c(0)
        out_j(1, 2); out_j(1, 3)
        rs_c(1); ln_c(1)
        # last query range is split in two so chunk 2's RS overlaps the
        # second half's attention, leaving only chunk 3's RS exposed
        run_qc(1536, 256)
        out_j(2, 2); out_j(2, 3)
        rs_c(2); ln_c(2)
        qdn_b = adram.tile([8, 512], F32, tag="qdn", name="qdn", bufs=2)
        aus_b = [attn_chunk(t, 1792, 256, 16, qdn_b) for t in range(4)]
        with tc.high_priority(offset=300):
            norm_qc(1792, 256, qdn_b, aus_b)
            out_j(3, 2); out_j(3, 3)
        rs_c(3); ln_c(3)
        fin.close()
        attn_ctx.close()
    nc.compile()
    return nc


def _build_cmask():
    # the 128x128 causal triangle (k <= q), duplicated for the two packed
    # heads -> [128, 256]
    k = np.arange(128)[:, None]
    q = np.arange(128)[None, :]
    m = (k <= q).astype(np.float16)
    return np.concatenate([m, m], axis=1)


def _make_in_maps(x0, W_in, W_o):
    x0 = np.asarray(x0, np.float32)
    W_in = np.asarray(W_in, np.float32)
    W_o = np.asarray(W_o, np.float32)
    wo16 = W_o.astype(np.float16)
    cmask = _build_cmask()
    in_maps = []
    for core in range(NCORES):
        bi, half = core // 2, core % 2
        hs = range(half * HL, half * HL + HL)
        wqk = np.concatenate(
            [W_in[:, h * 3 * DH: h * 3 * DH + DH] for h in hs]
            + [W_in[:, h * 3 * DH + DH: h * 3 * DH + 2 * DH] for h in hs], axis=1)
        wv = np.concatenate(
            [W_in[:, h * 3 * DH + 2 * DH: h * 3 * DH + 3 * DH] for h in hs], axis=1)
        in_maps.append(dict(
            xT=np.ascontiguousarray(x0[bi].T).astype(np.float16),
            wqk=np.ascontiguousarray(wqk).astype(np.float16),
            wv=np.ascontiguousarray(wv).astype(np.float16),
            wo=np.ascontiguousarray(wo16[half * HL * DH:(half + 1) * HL * DH]),
            xres=np.ascontiguousarray(x0[bi, half * SH:(half + 1) * SH]),
            cmask=cmask))
    return in_maps


_NC = None


def _run(x0, W_in, W_o, **run_kwargs):
    global _NC
    if _NC is None:
        _NC = build_nc()
    in_maps = _make_in_maps(x0, W_in, W_o)
    return run_bass_kernel_spmd(_NC, in_maps, list(range(NCORES)), **run_kwargs)


def kernel(x0, W_in, W_o, src_mask=None):
    res = _run(x0, W_in, W_o).results
    out = np.empty((B, S, D), np.float32)
    for core in range(NCORES):
        bi, half = core // 2, core % 2
        out[bi, half * SH:(half + 1) * SH] = res[core]["out"]
    return out


# revision 36
# speedup vs baseline: 1.0580x; 1.0580x over previous
"""Trainium2 Bass kernel for a causal dense-transformer attention layer.

Reference computation (b=4, s=2048, d=1024, 16 heads, dh=64):
  qkv = x0 @ W_in ; causal softmax attention ; out = attn @ W_o
  y = LayerNorm(out + x0)   (no affine, eps=1e-5)

Sharding over 8 cores: core = (batch bi = core//2, head-group tp = core%2).
Each core computes QKV projection + attention for its 8 heads of one batch
(tensor parallel over head groups), then the output projection partial sums
are pair-ReduceScattered so residual + LayerNorm run locally on each core's
1024 output rows.

v2 layout notes:
- scores are computed transposed (keys on partitions, queries free); both
  heads of a pair write adjacent PSUM banks of one [128,1024] tile so a
  single ACT exp instruction covers them (the Scalar engine is the pacing
  resource in the attention phase).
- softmax denominators ride as a ones-column inside V (PSUM row 64); they
  are normalized once per query-chunk: one batched reciprocal on an [8,512]
  gather, broadcast back via DRAM-roundtrip gpsimd DMAs.
- program order starts attention for head-pair 0 right after its Q/K
  projection so exp starts ~25us in; out-projection blocks are issued per
  128-row block as soon as their query-chunk is normalized, leaving only
  two small ReduceScatters exposed at the end.
"""

import os
import sys
from contextlib import ExitStack

import numpy as np

for _p in ("/opt/trn_rl_repo",):
    if os.path.isdir(_p) and _p not in sys.path:
        sys.path.insert(0, _p)

import concourse.bass as bass
import concourse.tile as tile
from concourse import bacc
from concourse import mybir
from concourse.bass_utils import run_bass_kernel_spmd

B, S, D = 4, 2048, 1024
NH, DH = 16, 64
HL = NH // 2          # heads per core
SH = S // 2           # output seq rows per core
NCORES = 8
SCALE = DH ** -0.5    # 0.125
LN_EPS = 1e-5

F16 = mybir.dt.float16
F32 = mybir.dt.float32
Exp = mybir.ActivationFunctionType.Exp
Ln = mybir.ActivationFunctionType.Ln

# out-proj chunk c -> the four 128-row q blocks it carries (2 low, 2 high)
CHUNKS = [[0, 1, 8, 9], [2, 3, 10, 11], [4, 5, 12, 13], [6, 7, 14, 15]]


def build_nc():
    nc = bacc.Bacc("TRN2", target_bir_lowering=False, num_devices=NCORES)
    xT = nc.declare_dram_parameter("xT", [D, S], F16, isOutput=False)
    wqk = nc.declare_dram_parameter("wqk", [D, 2 * HL * DH], F16, isOutput=False)
    wv = nc.declare_dram_parameter("wv", [D, HL * DH], F16, isOutput=False)
    wo = nc.declare_dram_parameter("wo", [HL * DH, D], F16, isOutput=False)
    xres = nc.declare_dram_parameter("xres", [SH, D], F32, isOutput=False)
    # [T | T]: the 128x128 causal triangle (k<=q), duplicated for both heads
    cmsk = nc.declare_dram_parameter("cmask", [128, 256], F16, isOutput=False)
    out = nc.declare_dram_parameter("out", [SH, D], F32, isOutput=True)

    with tile.TileContext(nc, num_cores=NCORES) as tc, ExitStack() as top:
        persist = top.enter_context(tc.tile_pool(name="persist", bufs=1))
        # QT rows 0..511 (tiles 0-3, head pair t on tile t), KT rows 512..1023
        qkt = [persist.tile([128, S], F16, name=f"qkt{m}") for m in range(8)]
        # V in (seq-part, head*dh free) orientation + trailing ones column
        vsb = [persist.tile([128, HL * (DH + 1)], F16, name=f"vsb{m}") for m in range(16)]
        # normalized attn-out^T (head*dh on partitions, seq free)
        aot = [persist.tile([128, S], F16, name=f"aot{t}") for t in range(4)]
        cm = persist.tile([128, 256], F16, name="cm")
        eps_t = persist.tile([128, 1], F32, name="eps_t")
        nc.vector.memset(eps_t, LN_EPS)
        for m in range(16):
            vones = vsb[m].rearrange("p (h c) -> p h c", c=DH + 1)[:, :, DH:DH + 1]
            nc.vector.memset(vones, 1.0)
        nc.sync.dma_start(out=cm, in_=cmsk[:, :])

        # attention pools open first so their PSUM banks never alias the
        # projection PSUM pool (pools are a strict stack; proj closes first)
        attn_ctx = ExitStack()
        adram = attn_ctx.enter_context(tc.tile_pool(name="adram", bufs=2, space="DRAM"))
        asb = attn_ctx.enter_context(tc.tile_pool(name="asb", bufs=5))
        scps = attn_ctx.enter_context(tc.tile_pool(name="scps", bufs=2, space="PSUM"))
        avps = attn_ctx.enter_context(tc.tile_pool(name="avps", bufs=1, space="PSUM"))
        small = attn_ctx.enter_context(tc.tile_pool(name="small", bufs=2))

        proj_ctx = ExitStack()
        proj_in = proj_ctx.enter_context(tc.tile_pool(name="proj_in", bufs=1))
        pjps = proj_ctx.enter_context(tc.tile_pool(name="pjps", bufs=2, space="PSUM"))
        xt = [proj_in.tile([128, S], F16, name=f"xt{k}") for k in range(8)]
        wqs = [proj_in.tile([128, 2 * HL * DH], F16, name=f"wqs{k}") for k in range(8)]
        wvs = [proj_in.tile([128, HL * DH], F16, name=f"wvs{k}") for k in range(8)]
        # interleave so the k-accumulation stream can start on first arrivals
        # spread input loads across three DMA queues so the projection's
        # first k-slices arrive as early as possible
        for k in range(8):
            nc.sync.dma_start(out=xt[k], in_=xT[k * 128:(k + 1) * 128, :])
            nc.gpsimd.dma_start(out=wqs[k], in_=wqk[k * 128:(k + 1) * 128, :])
            nc.scalar.dma_start(out=wvs[k], in_=wv[k * 128:(k + 1) * 128, :])

        # ~3.5us of junk matmuls on the (tiny, early-loaded) mask tile trip
        # the HAM clock gate to full rate before the real lead-in arrives
        wup = pjps.tile([128, 512], F32, tag="pj", name="wup")
        for _ in range(30):
            nc.tensor.matmul(wup[:, 0:256], cm[:, 0:128], cm[:, 0:256],
                             start=True, stop=True)
        wud = small.tile([1, 4], F32, tag="wud", name="wud", bufs=2)
        nc.vector.tensor_copy(wud, wup[0:1, 0:4])

        def proj_v(m):
            ps = pjps.tile([128, 512], F32, tag="pj", name="pjv")
            for k in range(8):
                nc.tensor.matmul(ps, xt[k][:, m * 128:(m + 1) * 128], wvs[k],
                                 start=(k == 0), stop=(k == 7))
            vdst = vsb[m].rearrange("p (h c) -> p h c", c=DH + 1)[:, :, 0:DH]
            nc.vector.tensor_copy(vdst, ps.rearrange("p (h c) -> p h c", c=DH))

        def proj_qk_g(m, q4):  # one 512-column group of Q/K tile m
            ps = pjps.tile([128, 512], F32, tag="pj", name="pjqk")
            for k in range(8):
                nc.tensor.matmul(ps, wqs[k][:, m * 128:(m + 1) * 128],
                                 xt[k][:, q4 * 512:(q4 + 1) * 512],
                                 start=(k == 0), stop=(k == 7))
            nc.vector.tensor_copy(qkt[m][:, q4 * 512:(q4 + 1) * 512], ps)

        def proj_qk(m):
            for q4 in range(4):
                proj_qk_g(m, q4)

        # lead-in: only what attention chunk (t=0, qc=0) needs at normal
        # priority; the rest of the projection is demoted so the scheduler
        # treats it as gap-filler under the ACT-paced attention stream
        # (dependencies still pull each piece in before its consumer)
        proj_qk_g(0, 0); proj_qk_g(4, 0)
        proj_v(0); proj_v(1); proj_v(2); proj_v(3)
        with tc.high_priority(offset=-800):
            for q4 in range(1, 4):
                proj_qk_g(0, q4); proj_qk_g(4, q4)
            proj_qk(1); proj_qk(5)
            proj_v(4); proj_v(5); proj_v(6); proj_v(7)
            proj_qk(2); proj_qk(6)
            proj_v(8); proj_v(9); proj_v(10); proj_v(11)
            proj_qk(3); proj_qk(7)
            proj_v(12); proj_v(13); proj_v(14); proj_v(15)

        w = DH + 1

        cmh = cm.rearrange("p (h c) -> p h c", h=2)

        def attn_chunk(t, qlo, qw, nkb, qdn):
            q_t, k_t = qkt[t], qkt[4 + t]
            dlo = qlo // 128  # first diagonal key block index
            HP = 512          # head pitch: keeps all matmul PSUM dsts bank-aligned
            av = avps.tile([65, 2 * HP], F32, tag="av", name="av")
            for kb in range(nkb):
                ksl = slice(kb * 128, (kb + 1) * 128)
                r = kb - dlo
                v0 = 128 * r if r > 0 else 0  # first causally-valid column
                vw = qw - v0
                sp = scps.tile([128, 2 * HP], F32, tag="sp", name="sp")
                # scores^T = K^T.T @ Q^T; the two heads land in adjacent
                # PSUM banks and use disjoint PE row groups (h0 / h64);
                # diagonal blocks write only the valid columns, compacted to
                # the bank start (matmul PSUM dsts must be bank-aligned)
                nc.tensor.matmul(sp[:, 0:vw], k_t[0:64, ksl],
                                 q_t[0:64, qlo + v0:qlo + qw], start=True, stop=True)
                nc.tensor.matmul(sp[:, HP:HP + vw], k_t[64:128, ksl],
                                 q_t[64:128, qlo + v0:qlo + qw], start=True, stop=True)
                ep = asb.tile([128, 2 * HP], F16, tag="ep", name="ep", bufs=5)
                eph = ep.rearrange("p (h q) -> p h q", h=2)
                sph = sp.rearrange("p (h q) -> p h q", h=2)
                # exp de-compacts: reads [0:vw], writes at [v0:qw]
                nc.scalar.activation(eph[:, :, v0:qw], sph[:, :, 0:vw], Exp, scale=SCALE)
                if r >= 0:  # diagonal 128-col block: apply causal triangle
                    if v0 > 0:
                        nc.vector.memset(eph[:, :, 0:v0], 0.0)
                    nc.vector.tensor_mul(eph[:, :, v0:v0 + 128],
                                         eph[:, :, v0:v0 + 128], cmh)
                st, sp_ = (kb == 0), (kb == nkb - 1)
                # attn-out^T accumulation; V carries a trailing ones column,
                # so the softmax denominator accumulates into PSUM row 64
                nc.tensor.matmul(av[:, 0:qw], vsb[kb][:, (2 * t) * w:(2 * t + 1) * w],
                                 ep[:, 0:qw], start=st, stop=sp_)
                nc.tensor.matmul(av[:, HP:HP + qw],
                                 vsb[kb][:, (2 * t + 1) * w:(2 * t + 2) * w],
                                 ep[:, HP:HP + qw], start=st, stop=sp_)
            # drain PSUM: rows 0..63 data, row 64 denominator
            au = small.tile([128, 512], F32, tag="au", name="au", bufs=10)
            stg = small.tile([65, 512], F32, tag="stg", name="stg", bufs=3)
            nc.vector.tensor_copy(au[0:65, 0:qw], av[:, 0:qw])
            nc.vector.tensor_copy(stg[:, 0:qw], av[:, HP:HP + qw])
            nc.sync.dma_start(out=qdn[2 * t:2 * t + 1, 0:qw], in_=au[64:65, 0:qw])
            nc.sync.dma_start(out=qdn[2 * t + 1:2 * t + 2, 0:qw], in_=stg[64:65, 0:qw])
            nc.sync.dma_start(out=au[64:128, 0:qw], in_=stg[0:64, 0:qw])
            return au

        def norm_qc(qlo, qw, qdn, aus):
            # one reciprocal for all 8 denominator rows of this query chunk,
            # then broadcast across partitions via DRAM-roundtrip DMA
            dn = small.tile([8, 512], F32, tag="dn", name="dn", bufs=2)
            nc.gpsimd.dma_start(out=dn[:, 0:qw], in_=qdn[:, 0:qw])
            dnr = small.tile([8, 512], F32, tag="dnr", name="dnr", bufs=2)
            # ~51-ULP approx reciprocal: ~5x faster than reciprocal(), far
            # more accurate than the softmax denominator needs, and short
            # enough not to head-of-line-block the DVE queue
            nc.vector.reciprocal_approx_fast(out=dnr[:, 0:qw], in_=dn[:, 0:qw])
            rdn = adram.tile([8, 512], F32, tag="rdn", name="rdn", bufs=2)
            nc.sync.dma_start(out=rdn[:, 0:qw], in_=dnr[:, 0:qw])
            for t in range(4):
                rb = small.tile([128, 512], F32, tag="rb", name="rb", bufs=2)
                for j in range(2):
                    srow = rdn[2 * t + j:2 * t + j + 1, 0:qw]
                    bc = bass.AP(tensor=srow.tensor, offset=srow.offset,
                                 ap=[[0, 64], [1, qw]])
                    nc.gpsimd.dma_start(out=rb[j * 64:(j + 1) * 64, 0:qw], in_=bc)
                with tc.high_priority(offset=-400):
                    nc.vector.tensor_mul(aot[t][:, qlo:qlo + qw], aus[t][:, 0:qw],
                                         rb[:, 0:qw])

        def run_qc(qlo, qw):
            nkb = (qlo + qw) // 128
            qdn = adram.tile([8, 512], F32, tag="qdn", name="qdn", bufs=2)
            aus = [attn_chunk(t, qlo, qw, nkb, qdn) for t in range(4)]
            norm_qc(qlo, qw, qdn, aus)

        # qc0 and qc1 interleaved t-major: each head pair's Q/K projection
        # unlock feeds 12 key-blocks of exp work instead of 4, so the ACT
        # stream stays fed while the demoted projection fills PE gaps
        qdn0 = adram.tile([8, 512], F32, tag="qdn", name="qdn", bufs=2)
        qdn1 = adram.tile([8, 512], F32, tag="qdn", name="qdn", bufs=2)
        aus0, aus1 = [], []
        for t in range(4):
            aus0.append(attn_chunk(t, 0, 512, 4, qdn0))
            aus1.append(attn_chunk(t, 512, 512, 8, qdn1))
        norm_qc(0, 512, qdn0, aus0)
        norm_qc(512, 512, qdn1, aus1)
        proj_ctx.close()

        fin = ExitStack()
        dpool = fin.enter_context(tc.tile_pool(name="dram", bufs=1, space="DRAM"))
        fsb = fin.enter_context(tc.tile_pool(name="fsb", bufs=1))
        fps = fin.enter_context(tc.tile_pool(name="fps", bufs=2, space="PSUM"))
        lnp = fin.enter_context(tc.tile_pool(name="lnp", bufs=2))

        wos = [fsb.tile([128, D], F16, name=f"wos{k}") for k in range(4)]
        for k in range(4):
            nc.sync.dma_start(out=wos[k], in_=wo[k * 128:(k + 1) * 128, :])

        rs_in = [dpool.tile([512, D], F16, name=f"rs_in{c}", bufs=4) for c in range(4)]
        rs_out = [dpool.tile([256, D], F16, name=f"rs_out{c}", bufs=4) for c in range(4)]

        def out_j(c, j):
            m = CHUNKS[c][j]
            pstg = lnp.tile([128, D], F16, tag="pstg", name="pstg")
            for n2 in range(2):
                po = fps.tile([128, 512], F32, tag="po", name="po")
                for k in range(4):
                    nc.tensor.matmul(po, aot[k][:, m * 128:(m + 1) * 128],
                                     wos[k][:, n2 * 512:(n2 + 1) * 512],
                                     start=(k == 0), stop=(k == 3))
                nc.vector.tensor_copy(pstg[:, n2 * 512:(n2 + 1) * 512], po)
            nc.sync.dma_start(out=rs_in[c][j * 128:(j + 1) * 128, :], in_=pstg)

        def rs_c(c):
            nc.gpsimd.collective_compute(
                "ReduceScatter", mybir.AluOpType.add,
                replica_groups=[[0, 1], [2, 3], [4, 5], [6, 7]],
                ins=[rs_in[c].opt()], outs=[rs_out[c].opt()])

        def ln_c(c):
            for j in range(2):
                m = 2 * c + j
                y = lnp.tile([128, D], F32, tag="y", name="y")
                yin = lnp.tile([128, D], F16, tag="yin", name="yin")
                xr = lnp.tile([128, D], F32, tag="xr", name="xr")
                nc.sync.dma_start(out=xr, in_=xres[m * 128:(m + 1) * 128, :])
                nc.gpsimd.dma_start(out=yin, in_=rs_out[c][j * 128:(j + 1) * 128, :])
                nc.vector.tensor_add(y, yin, xr)
                stats = lnp.tile([128, 2, 6], F32, tag="st", name="st")
                mv = lnp.tile([128, 2], F32, tag="mv", name="mv")
                for sg in range(2):
                    nc.vector.bn_stats(out=stats[:, sg, :], in_=y[:, sg * 512:(sg + 1) * 512])
                nc.vector.bn_aggr(out=mv, in_=stats)
                # rstd = rsqrt(var+eps) via DVE-only Newton iteration (the
                # ACT sqrt lives in a different table set than exp, and each
                # switch evicts the attention exp tables for ~2.7us)
                vv = lnp.tile([128, 1], F32, tag="vv", name="vv")
                nc.vector.tensor_scalar(out=vv, in0=mv[:, 1:2], scalar1=LN_EPS,
                                        scalar2=None, op0=mybir.AluOpType.add)
                rstd = lnp.tile([128, 1], F32, tag="rs", name="rs")
                tn = lnp.tile([128, 1], F32, tag="tn", name="tn")
                nc.vector.reciprocal(rstd, vv)  # r = 1/v
                # seed 0.675*r + 0.3 (<=10% err for v in [0.95, 8.3]), then
                # 3x y *= 1.5 - 0.5*v*y^2 -> ~1e-7 rel err
                nc.vector.tensor_scalar(out=rstd, in0=rstd, scalar1=0.675,
                                        scalar2=0.3, op0=mybir.AluOpType.mult,
                                        op1=mybir.AluOpType.add)
                for _ in range(3):
                    nc.vector.tensor_mul(tn, rstd, rstd)
                    nc.vector.tensor_mul(tn, tn, vv)
                    nc.vector.tensor_scalar(out=tn, in0=tn, scalar1=-0.5,
                                            scalar2=1.5, op0=mybir.AluOpType.mult,
                                            op1=mybir.AluOpType.add)
                    nc.vector.tensor_mul(rstd, rstd, tn)
                ot = lnp.tile([128, D], F32, tag="ot", name="ot")
                nc.vector.tensor_scalar(out=ot, in0=y, scalar1=mv[:, 0:1], scalar2=rstd,
                                        op0=mybir.AluOpType.subtract,
                                        op1=mybir.AluOpType.mult)
                nc.sync.dma_start(out=out[m * 128:(m + 1) * 128, :], in_=ot)

        # q blocks 0..7 (chunk lows) can project as soon as qc0/qc1 land
        # q blocks 0..7 (chunk lows) fill PE gaps under qc2's attention
        with tc.high_priority(offset=-400):
            for c in range(4):
                out_j(c, 0)
                out_j(c, 1)
        run_qc(1024, 512)
        out_j(0, 2); out_j(0, 3)
        rs_c(0); ln_c(0)
        out_j(1, 2); out_j(1, 3)
        rs_c(1); ln_c(1)
        # last query range is split in two so chunk 2's RS overlaps the
        # second half's attention, leaving only chunk 3's RS exposed
        run_qc(1536, 256)
        out_j(2, 2); out_j(2, 3)
        rs_c(2); ln_c(2)
        qdn_b = adram.tile([8, 512], F32, tag="qdn", name="qdn", bufs=2)
        aus_b = [attn_chunk(t, 1792, 256, 16, qdn_b) for t in range(4)]
        with tc.high_priority(offset=300):
            norm_qc(1792, 256, qdn_b, aus_b)
            out_j(3, 2); out_j(3, 3)
        rs_c(3); ln_c(3)
        fin.close()
        attn_ctx.close()
    nc.compile()
    return nc


def _build_cmask():
    # the 128x128 causal triangle (k <= q), duplicated for the two packed
    # heads -> [128, 256]
    k = np.arange(128)[:, None]
    q = np.arange(128)[None, :]
    m = (k <= q).astype(np.float16)
    return np.concatenate([m, m], axis=1)


def _make_in_maps(x0, W_in, W_o):
    x0 = np.asarray(x0, np.float32)
    W_in = np.asarray(W_in, np.float32)
    W_o = np.asarray(W_o, np.float32)
    wo16 = W_o.astype(np.float16)
    cmask = _build_cmask()
    in_maps = []
    for core in range(NCORES):
        bi, half = core // 2, core % 2
        hs = range(half * HL, half * HL + HL)
        wqk = np.concatenate(
            [W_in[:, h * 3 * DH: h * 3 * DH + DH] for h in hs]
            + [W_in[:, h * 3 * DH + DH: h * 3 * DH + 2 * DH] for h in hs], axis=1)
        wv = np.concatenate(
            [W_in[:, h * 3 * DH + 2 * DH: h * 3 * DH + 3 * DH] for h in hs], axis=1)
        in_maps.append(dict(
            xT=np.ascontiguousarray(x0[bi].T).astype(np.float16),
            wqk=np.ascontiguousarray(wqk).astype(np.float16),
            wv=np.ascontiguousarray(wv).astype(np.float16),
            wo=np.ascontiguousarray(wo16[half * HL * DH:(half + 1) * HL * DH]),
            xres=np.ascontiguousarray(x0[bi, half * SH:(half + 1) * SH]),
            cmask=cmask))
    return in_maps


_NC = None


def _run(x0, W_in, W_o, **run_kwargs):
    global _NC
    if _NC is None:
        _NC = build_nc()
    in_maps = _make_in_maps(x0, W_in, W_o)
    return run_bass_kernel_spmd(_NC, in_maps, list(range(NCORES)), **run_kwargs)


def kernel(x0, W_in, W_o, src_mask=None):
    res = _run(x0, W_in, W_o).results
    out = np.empty((B, S, D), np.float32)
    for core in range(NCORES):
        bi, half = core // 2, core % 2
        out[bi, half * SH:(half + 1) * SH] = res[core]["out"]
    return out


# revision 38
# speedup vs baseline: 1.0769x; 1.0179x over previous
"""Trainium2 Bass kernel for a causal dense-transformer attention layer.

Reference computation (b=4, s=2048, d=1024, 16 heads, dh=64):
  qkv = x0 @ W_in ; causal softmax attention ; out = attn @ W_o
  y = LayerNorm(out + x0)   (no affine, eps=1e-5)

Sharding over 8 cores: core = (batch bi = core//2, head-group tp = core%2).
Each core computes QKV projection + attention for its 8 heads of one batch
(tensor parallel over head groups), then the output projection partial sums
are pair-ReduceScattered so residual + LayerNorm run locally on each core's
1024 output rows.

v2 layout notes:
- scores are computed transposed (keys on partitions, queries free); both
  heads of a pair write adjacent PSUM banks of one [128,1024] tile so a
  single ACT exp instruction covers them (the Scalar engine is the pacing
  resource in the attention phase).
- softmax denominators ride as a ones-column inside V (PSUM row 64); they
  are normalized once per query-chunk: one batched reciprocal on an [8,512]
  gather, broadcast back via DRAM-roundtrip gpsimd DMAs.
- program order starts attention for head-pair 0 right after its Q/K
  projection so exp starts ~25us in; out-projection blocks are issued per
  128-row block as soon as their query-chunk is normalized, leaving only
  two small ReduceScatters exposed at the end.
"""

import os
import sys
from contextlib import ExitStack

import numpy as np

for _p in ("/opt/trn_rl_repo",):
    if os.path.isdir(_p) and _p not in sys.path:
        sys.path.insert(0, _p)

import concourse.bass as bass
import concourse.tile as tile
from concourse import bacc
from concourse import mybir
from concourse.bass_utils import run_bass_kernel_spmd

B, S, D = 4, 2048, 1024
NH, DH = 16, 64
HL = NH // 2          # heads per core
SH = S // 2           # output seq rows per core
NCORES = 8
SCALE = DH ** -0.5    # 0.125
LN_EPS = 1e-5

F16 = mybir.dt.float16
F32 = mybir.dt.float32
Exp = mybir.ActivationFunctionType.Exp
Ln = mybir.ActivationFunctionType.Ln

# out-proj chunk c -> the four 128-row q blocks it carries (2 low, 2 high)
CHUNKS = [[0, 1, 8, 9], [2, 3, 10, 11], [4, 5, 12, 13], [6, 7, 14, 15]]


def build_nc():
    nc = bacc.Bacc("TRN2", target_bir_lowering=False, num_devices=NCORES)
    xT = nc.declare_dram_parameter("xT", [D, S], F16, isOutput=False)
    wqk = nc.declare_dram_parameter("wqk", [D, 2 * HL * DH], F16, isOutput=False)
    wv = nc.declare_dram_parameter("wv", [D, HL * DH], F16, isOutput=False)
    wo = nc.declare_dram_parameter("wo", [HL * DH, D], F16, isOutput=False)
    xres = nc.declare_dram_parameter("xres", [SH, D], F32, isOutput=False)
    # [T | T]: the 128x128 causal triangle (k<=q), duplicated for both heads
    cmsk = nc.declare_dram_parameter("cmask", [128, 256], F16, isOutput=False)
    out = nc.declare_dram_parameter("out", [SH, D], F32, isOutput=True)

    with tile.TileContext(nc, num_cores=NCORES) as tc, ExitStack() as top:
        persist = top.enter_context(tc.tile_pool(name="persist", bufs=1))
        # QT rows 0..511 (tiles 0-3, head pair t on tile t), KT rows 512..1023
        qkt = [persist.tile([128, S], F16, name=f"qkt{m}") for m in range(8)]
        # V in (seq-part, head*dh free) orientation + trailing ones column
        vsb = [persist.tile([128, HL * (DH + 1)], F16, name=f"vsb{m}") for m in range(16)]
        # normalized attn-out^T (head*dh on partitions, seq free)
        aot = [persist.tile([128, S], F16, name=f"aot{t}") for t in range(4)]
        cm = persist.tile([128, 256], F16, name="cm")
        eps_t = persist.tile([128, 1], F32, name="eps_t")
        nc.vector.memset(eps_t, LN_EPS)
        for m in range(16):
            vones = vsb[m].rearrange("p (h c) -> p h c", c=DH + 1)[:, :, DH:DH + 1]
            nc.vector.memset(vones, 1.0)
        nc.sync.dma_start(out=cm, in_=cmsk[:, :])

        # attention pools open first so their PSUM banks never alias the
        # projection PSUM pool (pools are a strict stack; proj closes first)
        attn_ctx = ExitStack()
        adram = attn_ctx.enter_context(tc.tile_pool(name="adram", bufs=2, space="DRAM"))
        asb = attn_ctx.enter_context(tc.tile_pool(name="asb", bufs=5))
        scps = attn_ctx.enter_context(tc.tile_pool(name="scps", bufs=2, space="PSUM"))
        avps = attn_ctx.enter_context(tc.tile_pool(name="avps", bufs=1, space="PSUM"))
        small = attn_ctx.enter_context(tc.tile_pool(name="small", bufs=2))

        proj_ctx = ExitStack()
        proj_in = proj_ctx.enter_context(tc.tile_pool(name="proj_in", bufs=1))
        pjps = proj_ctx.enter_context(tc.tile_pool(name="pjps", bufs=2, space="PSUM"))
        xt = [proj_in.tile([128, S], F16, name=f"xt{k}") for k in range(8)]
        wqs = [proj_in.tile([128, 2 * HL * DH], F16, name=f"wqs{k}") for k in range(8)]
        wvs = [proj_in.tile([128, HL * DH], F16, name=f"wvs{k}") for k in range(8)]
        # interleave so the k-accumulation stream can start on first arrivals
        # spread input loads across three DMA queues so the projection's
        # first k-slices arrive as early as possible
        for k in range(8):
            nc.sync.dma_start(out=xt[k], in_=xT[k * 128:(k + 1) * 128, :])
            nc.gpsimd.dma_start(out=wqs[k], in_=wqk[k * 128:(k + 1) * 128, :])
            nc.scalar.dma_start(out=wvs[k], in_=wv[k * 128:(k + 1) * 128, :])

        # ~3.5us of junk matmuls on the (tiny, early-loaded) mask tile trip
        # the HAM clock gate to full rate before the real lead-in arrives
        wup = pjps.tile([128, 512], F32, tag="pj", name="wup")
        for _ in range(30):
            nc.tensor.matmul(wup[:, 0:256], cm[:, 0:128], cm[:, 0:256],
                             start=True, stop=True)
        wud = small.tile([1, 4], F32, tag="wud", name="wud", bufs=2)
        nc.vector.tensor_copy(wud, wup[0:1, 0:4])

        def proj_v(m):
            ps = pjps.tile([128, 512], F32, tag="pj", name="pjv")
            for k in range(8):
                nc.tensor.matmul(ps, xt[k][:, m * 128:(m + 1) * 128], wvs[k],
                                 start=(k == 0), stop=(k == 7))
            vdst = vsb[m].rearrange("p (h c) -> p h c", c=DH + 1)[:, :, 0:DH]
            nc.vector.tensor_copy(vdst, ps.rearrange("p (h c) -> p h c", c=DH))

        def proj_qk_g(m, q4):  # one 512-column group of Q/K tile m
            ps = pjps.tile([128, 512], F32, tag="pj", name="pjqk")
            for k in range(8):
                nc.tensor.matmul(ps, wqs[k][:, m * 128:(m + 1) * 128],
                                 xt[k][:, q4 * 512:(q4 + 1) * 512],
                                 start=(k == 0), stop=(k == 7))
            nc.vector.tensor_copy(qkt[m][:, q4 * 512:(q4 + 1) * 512], ps)

        def proj_qk(m):
            for q4 in range(4):
                proj_qk_g(m, q4)

        # lead-in: only what attention chunk (t=0, qc=0) needs at normal
        # priority; the rest of the projection is demoted so the scheduler
        # treats it as gap-filler under the ACT-paced attention stream
        # (dependencies still pull each piece in before its consumer)
        proj_qk_g(0, 0); proj_qk_g(4, 0)
        proj_v(0); proj_v(1); proj_v(2); proj_v(3)
        with tc.high_priority(offset=-800):
            for q4 in range(1, 4):
                proj_qk_g(0, q4); proj_qk_g(4, q4)
            proj_qk(1); proj_qk(5)
            proj_v(4); proj_v(5); proj_v(6); proj_v(7)
            proj_qk(2); proj_qk(6)
            proj_v(8); proj_v(9); proj_v(10); proj_v(11)
            proj_qk(3); proj_qk(7)
            proj_v(12); proj_v(13); proj_v(14); proj_v(15)

        w = DH + 1

        cmh = cm.rearrange("p (h c) -> p h c", h=2)

        def attn_chunk(t, qlo, qw, nkb, qdn, row0=None):
            r0 = 2 * t if row0 is None else row0
            q_t, k_t = qkt[t], qkt[4 + t]
            dlo = qlo // 128  # first diagonal key block index
            HP = 512          # head pitch: keeps all matmul PSUM dsts bank-aligned
            av = avps.tile([65, 2 * HP], F32, tag="av", name="av")
            for kb in range(nkb):
                ksl = slice(kb * 128, (kb + 1) * 128)
                r = kb - dlo
                v0 = 128 * r if r > 0 else 0  # first causally-valid column
                vw = qw - v0
                sp = scps.tile([128, 2 * HP], F32, tag="sp", name="sp")
                # scores^T = K^T.T @ Q^T; the two heads land in adjacent
                # PSUM banks and use disjoint PE row groups (h0 / h64);
                # diagonal blocks write only the valid columns, compacted to
                # the bank start (matmul PSUM dsts must be bank-aligned)
                nc.tensor.matmul(sp[:, 0:vw], k_t[0:64, ksl],
                                 q_t[0:64, qlo + v0:qlo + qw], start=True, stop=True)
                nc.tensor.matmul(sp[:, HP:HP + vw], k_t[64:128, ksl],
                                 q_t[64:128, qlo + v0:qlo + qw], start=True, stop=True)
                ep = asb.tile([128, 2 * HP], F16, tag="ep", name="ep", bufs=5)
                eph = ep.rearrange("p (h q) -> p h q", h=2)
                sph = sp.rearrange("p (h q) -> p h q", h=2)
                # exp de-compacts: reads [0:vw], writes at [v0:qw]
                nc.scalar.activation(eph[:, :, v0:qw], sph[:, :, 0:vw], Exp, scale=SCALE)
                if r >= 0:  # diagonal 128-col block: apply causal triangle
                    if v0 > 0:
                        nc.vector.memset(eph[:, :, 0:v0], 0.0)
                    nc.vector.tensor_mul(eph[:, :, v0:v0 + 128],
                                         eph[:, :, v0:v0 + 128], cmh)
                st, sp_ = (kb == 0), (kb == nkb - 1)
                # attn-out^T accumulation; V carries a trailing ones column,
                # so the softmax denominator accumulates into PSUM row 64
                nc.tensor.matmul(av[:, 0:qw], vsb[kb][:, (2 * t) * w:(2 * t + 1) * w],
                                 ep[:, 0:qw], start=st, stop=sp_)
                nc.tensor.matmul(av[:, HP:HP + qw],
                                 vsb[kb][:, (2 * t + 1) * w:(2 * t + 2) * w],
                                 ep[:, HP:HP + qw], start=st, stop=sp_)
            # drain PSUM: rows 0..63 data, row 64 denominator
            au = small.tile([128, 512], F32, tag="au", name="au", bufs=10)
            stg = small.tile([65, 512], F32, tag="stg", name="stg", bufs=3)
            nc.vector.tensor_copy(au[0:65, 0:qw], av[:, 0:qw])
            nc.vector.tensor_copy(stg[:, 0:qw], av[:, HP:HP + qw])
            nc.sync.dma_start(out=qdn[r0:r0 + 1, 0:qw], in_=au[64:65, 0:qw])
            nc.sync.dma_start(out=qdn[r0 + 1:r0 + 2, 0:qw], in_=stg[64:65, 0:qw])
            nc.sync.dma_start(out=au[64:128, 0:qw], in_=stg[0:64, 0:qw])
            return au

        def norm_qc(qlo, qw, qdn, aus):
            # one reciprocal for all 8 denominator rows of this query chunk,
            # then broadcast across partitions via DRAM-roundtrip DMA
            dn = small.tile([8, 512], F32, tag="dn", name="dn", bufs=2)
            nc.gpsimd.dma_start(out=dn[:, 0:qw], in_=qdn[:, 0:qw])
            dnr = small.tile([8, 512], F32, tag="dnr", name="dnr", bufs=2)
            # ~51-ULP approx reciprocal: ~5x faster than reciprocal(), far
            # more accurate than the softmax denominator needs, and short
            # enough not to head-of-line-block the DVE queue
            nc.vector.reciprocal_approx_fast(out=dnr[:, 0:qw], in_=dn[:, 0:qw])
            rdn = adram.tile([8, 512], F32, tag="rdn", name="rdn", bufs=2)
            nc.sync.dma_start(out=rdn[:, 0:qw], in_=dnr[:, 0:qw])
            for t in range(4):
                rb = small.tile([128, 512], F32, tag="rb", name="rb", bufs=2)
                for j in range(2):
                    srow = rdn[2 * t + j:2 * t + j + 1, 0:qw]
                    bc = bass.AP(tensor=srow.tensor, offset=srow.offset,
                                 ap=[[0, 64], [1, qw]])
                    nc.gpsimd.dma_start(out=rb[j * 64:(j + 1) * 64, 0:qw], in_=bc)
                with tc.high_priority(offset=-400):
                    nc.vector.tensor_mul(aot[t][:, qlo:qlo + qw], aus[t][:, 0:qw],
                                         rb[:, 0:qw])

        def norm_one(t, qlo, qw, qdn2, au):
            # single-chunk normalization: short chain so the tail's out-proj
            # and ReduceScatter launch as early as possible
            dn2 = small.tile([2, 512], F32, tag="dn2", name="dn2", bufs=4)
            nc.gpsimd.dma_start(out=dn2[:, 0:qw], in_=qdn2[0:2, 0:qw])
            dnr2 = small.tile([2, 512], F32, tag="dnr2", name="dnr2", bufs=4)
            nc.vector.reciprocal_approx_fast(out=dnr2[:, 0:qw], in_=dn2[:, 0:qw])
            rdn2 = adram.tile([2, 512], F32, tag="rdn", name="rdn2", bufs=2)
            nc.sync.dma_start(out=rdn2[:, 0:qw], in_=dnr2[:, 0:qw])
            rb = small.tile([128, 512], F32, tag="rb", name="rb", bufs=2)
            for j in range(2):
                srow = rdn2[j:j + 1, 0:qw]
                bc = bass.AP(tensor=srow.tensor, offset=srow.offset,
                             ap=[[0, 64], [1, qw]])
                nc.gpsimd.dma_start(out=rb[j * 64:(j + 1) * 64, 0:qw], in_=bc)
            nc.vector.tensor_mul(aot[t][:, qlo:qlo + qw], au[:, 0:qw], rb[:, 0:qw])

        def run_qc(qlo, qw):
            nkb = (qlo + qw) // 128
            qdn = adram.tile([8, 512], F32, tag="qdn", name="qdn", bufs=2)
            aus = [attn_chunk(t, qlo, qw, nkb, qdn) for t in range(4)]
            norm_qc(qlo, qw, qdn, aus)

        # qc0 and qc1 interleaved t-major: each head pair's Q/K projection
        # unlock feeds 12 key-blocks of exp work instead of 4, so the ACT
        # stream stays fed while the demoted projection fills PE gaps
        qdn0 = adram.tile([8, 512], F32, tag="qdn", name="qdn", bufs=2)
        qdn1 = adram.tile([8, 512], F32, tag="qdn", name="qdn", bufs=2)
        aus0, aus1 = [], []
        for t in range(4):
            aus0.append(attn_chunk(t, 0, 512, 4, qdn0))
            aus1.append(attn_chunk(t, 512, 512, 8, qdn1))
        norm_qc(0, 512, qdn0, aus0)
        norm_qc(512, 512, qdn1, aus1)
        proj_ctx.close()

        fin = ExitStack()
        dpool = fin.enter_context(tc.tile_pool(name="dram", bufs=1, space="DRAM"))
        fsb = fin.enter_context(tc.tile_pool(name="fsb", bufs=1))
        fps = fin.enter_context(tc.tile_pool(name="fps", bufs=2, space="PSUM"))
        lnp = fin.enter_context(tc.tile_pool(name="lnp", bufs=2))

        wos = [fsb.tile([128, D], F16, name=f"wos{k}") for k in range(4)]
        for k in range(4):
            nc.sync.dma_start(out=wos[k], in_=wo[k * 128:(k + 1) * 128, :])

        rs_in = [dpool.tile([512, D], F16, name=f"rs_in{c}", bufs=4) for c in range(4)]
        rs_out = [dpool.tile([256, D], F16, name=f"rs_out{c}", bufs=4) for c in range(4)]

        def out_j(c, j):
            m = CHUNKS[c][j]
            pstg = lnp.tile([128, D], F16, tag="pstg", name="pstg")
            for n2 in range(2):
                po = fps.tile([128, 512], F32, tag="po", name="po")
                for k in range(4):
                    nc.tensor.matmul(po, aot[k][:, m * 128:(m + 1) * 128],
                                     wos[k][:, n2 * 512:(n2 + 1) * 512],
                                     start=(k == 0), stop=(k == 3))
                nc.vector.tensor_copy(pstg[:, n2 * 512:(n2 + 1) * 512], po)
            nc.sync.dma_start(out=rs_in[c][j * 128:(j + 1) * 128, :], in_=pstg)

        def rs_c(c):
            nc.gpsimd.collective_compute(
                "ReduceScatter", mybir.AluOpType.add,
                replica_groups=[[0, 1], [2, 3], [4, 5], [6, 7]],
                ins=[rs_in[c].opt()], outs=[rs_out[c].opt()])

        def ln_c(c):
            for j in range(2):
                m = 2 * c + j
                y = lnp.tile([128, D], F32, tag="y", name="y")
                yin = lnp.tile([128, D], F16, tag="yin", name="yin")
                xr = lnp.tile([128, D], F32, tag="xr", name="xr")
                nc.sync.dma_start(out=xr, in_=xres[m * 128:(m + 1) * 128, :])
                nc.gpsimd.dma_start(out=yin, in_=rs_out[c][j * 128:(j + 1) * 128, :])
                nc.vector.tensor_add(y, yin, xr)
                stats = lnp.tile([128, 2, 6], F32, tag="st", name="st")
                mv = lnp.tile([128, 2], F32, tag="mv", name="mv")
                for sg in range(2):
                    nc.vector.bn_stats(out=stats[:, sg, :], in_=y[:, sg * 512:(sg + 1) * 512])
                nc.vector.bn_aggr(out=mv, in_=stats)
                # rstd = rsqrt(var+eps) via DVE-only Newton iteration (the
                # ACT sqrt lives in a different table set than exp, and each
                # switch evicts the attention exp tables for ~2.7us)
                vv = lnp.tile([128, 1], F32, tag="vv", name="vv")
                nc.vector.tensor_scalar(out=vv, in0=mv[:, 1:2], scalar1=LN_EPS,
                                        scalar2=None, op0=mybir.AluOpType.add)
                rstd = lnp.tile([128, 1], F32, tag="rs", name="rs")
                tn = lnp.tile([128, 1], F32, tag="tn", name="tn")
                nc.vector.reciprocal(rstd, vv)  # r = 1/v
                # seed 0.675*r + 0.3 (<=10% err for v in [0.95, 8.3]), then
                # 3x y *= 1.5 - 0.5*v*y^2 -> ~1e-7 rel err
                nc.vector.tensor_scalar(out=rstd, in0=rstd, scalar1=0.675,
                                        scalar2=0.3, op0=mybir.AluOpType.mult,
                                        op1=mybir.AluOpType.add)
                for _ in range(3):
                    nc.vector.tensor_mul(tn, rstd, rstd)
                    nc.vector.tensor_mul(tn, tn, vv)
                    nc.vector.tensor_scalar(out=tn, in0=tn, scalar1=-0.5,
                                            scalar2=1.5, op0=mybir.AluOpType.mult,
                                            op1=mybir.AluOpType.add)
                    nc.vector.tensor_mul(rstd, rstd, tn)
                ot = lnp.tile([128, D], F32, tag="ot", name="ot")
                nc.vector.tensor_scalar(out=ot, in0=y, scalar1=mv[:, 0:1], scalar2=rstd,
                                        op0=mybir.AluOpType.subtract,
                                        op1=mybir.AluOpType.mult)
                nc.sync.dma_start(out=out[m * 128:(m + 1) * 128, :], in_=ot)

        # q blocks 0..7 (chunk lows) can project as soon as qc0/qc1 land
        # q blocks 0..7 (chunk lows) fill PE gaps under qc2's attention
        with tc.high_priority(offset=-400):
            for c in range(4):
                out_j(c, 0)
                out_j(c, 1)
        run_qc(1024, 512)
        out_j(0, 2); out_j(0, 3)
        rs_c(0); ln_c(0)
        out_j(1, 2); out_j(1, 3)
        rs_c(1); ln_c(1)
        # last query range is split in two so chunk 2's RS overlaps the
        # second half's attention, leaving only chunk 3's RS exposed
        for t in range(4):
            qdn2 = adram.tile([8, 512], F32, tag="qdn", name="qdn2", bufs=2)
            au = attn_chunk(t, 1536, 256, 14, qdn2, row0=0)
            norm_one(t, 1536, 256, qdn2, au)
        out_j(2, 2); out_j(2, 3)
        rs_c(2); ln_c(2)
        for t in range(4):
            qdn2 = adram.tile([8, 512], F32, tag="qdn", name="qdn2", bufs=2)
            au = attn_chunk(t, 1792, 256, 16, qdn2, row0=0)
            norm_one(t, 1792, 256, qdn2, au)
        out_j(3, 2); out_j(3, 3)
        rs_c(3); ln_c(3)
        fin.close()
        attn_ctx.close()
    nc.compile()
    return nc


def _build_cmask():
    # the 128x128 causal triangle (k <= q), duplicated for the two packed
    # heads -> [128, 256]
    k = np.arange(128)[:, None]
    q = np.arange(128)[None, :]
    m = (k <= q).astype(np.float16)
    return np.concatenate([m, m], axis=1)


def _make_in_maps(x0, W_in, W_o):
    x0 = np.asarray(x0, np.float32)
    W_in = np.asarray(W_in, np.float32)
    W_o = np.asarray(W_o, np.float32)
    wo16 = W_o.astype(np.float16)
    cmask = _build_cmask()
    in_maps = []
    for core in range(NCORES):
        bi, half = core // 2, core % 2
        hs = range(half * HL, half * HL + HL)
        wqk = np.concatenate(
            [W_in[:, h * 3 * DH: h * 3 * DH + DH] for h in hs]
            + [W_in[:, h * 3 * DH + DH: h * 3 * DH + 2 * DH] for h in hs], axis=1)
        wv = np.concatenate(
            [W_in[:, h * 3 * DH + 2 * DH: h * 3 * DH + 3 * DH] for h in hs], axis=1)
        in_maps.append(dict(
            xT=np.ascontiguousarray(x0[bi].T).astype(np.float16),
            wqk=np.ascontiguousarray(wqk).astype(np.float16),
            wv=np.ascontiguousarray(wv).astype(np.float16),
            wo=np.ascontiguousarray(wo16[half * HL * DH:(half + 1) * HL * DH]),
            xres=np.ascontiguousarray(x0[bi, half * SH:(half + 1) * SH]),
            cmask=cmask))
    return in_maps


_NC = None


def _run(x0, W_in, W_o, **run_kwargs):
    global _NC
    if _NC is None:
        _NC = build_nc()
    in_maps = _make_in_maps(x0, W_in, W_o)
    return run_bass_kernel_spmd(_NC, in_maps, list(range(NCORES)), **run_kwargs)


def kernel(x0, W_in, W_o, src_mask=None):
    res = _run(x0, W_in, W_o).results
    out = np.empty((B, S, D), np.float32)
    for core in range(NCORES):
        bi, half = core // 2, core % 2
        out[bi, half * SH:(half + 1) * SH] = res[core]["out"]
    return out
